# revision 6
# baseline (speedup 1.0000x reference)
"""BitNet attention (GQA, 32 q-heads / 8 kv-heads, hidden 4096, seq 2048) on 8
Trainium2 NeuronCores — fp8 DoubleRow edition.

Sharding: tensor-parallel over heads (core i: q-heads 4i..4i+3, kv-head i,
o_proj contribution of its 512 hidden columns; host sums the 8 partials).

Main idea vs the bf16 version: BitNet weights are exactly +-1, which fp8-e4m3
represents exactly, and the TRN2 PE runs fp8 matmuls in DoubleRow perf mode at
2x bf16 throughput (K=256 per instruction).  Quantization noise is held inside
the rel-err budget by exploiting the structure of the problem:
  - scores are O(0.25) and the softmax nearly uniform, so fp8 noise on x
    entering Q/K (weights exact) is strongly suppressed through the softmax;
    K additionally runs a 2-pass (x + residual/16) split which is free vs
    bf16 (K proj is 8x smaller than Q).
  - V runs the 2-pass split too (full bf16-level precision at fp8 speed).
  - the attention-value product uses *centered* probabilities:  P@v =
    sum(v) + (P-1)@v.  P-1 is O(0.25) so its fp8 quantization error is ~4x
    smaller than quantizing P~1 directly; the exact rank-1 sum(v) term is
    restored via a per-partition ACT bias.
  - o_proj is centered the same way: r = attn*256/E - m (m = mean attention
    output, a per-d constant) is quantized fp8; the dominant rank-1 term
    W_o @ m is added back exactly via replicated-m DoubleRow matmuls
    (m split into fp8 m1 + residual, both exact on the PE).
  - softmax denominator E = 2048 + rowsum(P-1) comes from DoubleRow rowsums
    with a [128,2,4]-ones stationary; 1/E from the ACT Reciprocal table
    (bias=+2048 fused), optionally polished by one DVE Newton step.
  - the per-query 1/E scale is applied through a rank-1 outer product
    materialized from a bf16 hi/lo split (f32-accurate), as PE work.
Outputs are scaled by 256 on-device (fp8 subnormal avoidance for r); the host
divides by 256 when applying the final o-scales.
"""

import numpy as np
import ml_dtypes

import concourse.bass as bass
import concourse.mybir as mybir
import concourse.tile as tile
from concourse.vector_clock import ScopedClock
from concourse.bass_utils import run_bass_kernel_spmd

F32 = mybir.dt.float32
BF16 = mybir.dt.bfloat16
FP8 = mybir.dt.float8e4
NPF8 = ml_dtypes.float8_e4m3
NPBF = ml_dtypes.bfloat16

HIDDEN = 4096
T = 2048
N_CORES = 8
FQ = HIDDEN // N_CORES   # 512 q-features per core
H = 4                    # q heads per core
DH = 128                 # head dim
CC = 16                  # 256-wide contraction chunks (fp8 DoubleRow)
TQ = 4                   # token quarters (512 tokens each)
KT = 16                  # key tiles of 128
QB = 4                   # query blocks of 512
ALU = mybir.AluOpType
AF = mybir.ActivationFunctionType
DR = mybir.MatmulPerfMode.DoubleRow

TRACE = False            # set by test.py for profiling runs
TRACE_ALL_CORES = False

RECIP_NEWTON = False     # ACT-table 1/E measured at 1.16e-5 rel err

_MAX_DRAIN_WAITS = 1
_MAX_INST_WAITS = 1


def _split_sync_waits(nc):
    """The walrus build in this container rejects instructions carrying more
    than one sync wait ("Too many sync wait commands"). Cap every instruction
    at _MAX_INST_WAITS waits; spill the excess onto InstEventSemaphore
    (standalone wait) instructions inserted immediately before on the same
    engine (engines are in-order, so combined wait semantics are identical)."""
    counter = [0]

    def _mk_wait(engine, waits):
        counter[0] += 1
        nop = mybir.InstEventSemaphore(
            name=f"waitsplit_{counter[0]}", ins=[], outs=[]
        )
        nop.engine = engine
        nop.sync_info = mybir.SyncInfo(on_wait=list(waits), on_update=[])
        nc.register_instruction(nop, overwrite=True)
        return nop

    for bb in nc.main_func.blocks:
        insts = list(bb.instructions)
        out = []
        changed = False
        for ins in insts:
            si = ins.sync_info
            waits = list(si.on_wait or []) if si else []
            if len(waits) > _MAX_INST_WAITS:
                changed = True
                rest = waits[:-_MAX_INST_WAITS]
                for i in range(0, len(rest), _MAX_INST_WAITS):
                    out.append(_mk_wait(ins.engine, rest[i : i + _MAX_INST_WAITS]))
                ins.sync_info = mybir.SyncInfo(
                    on_wait=waits[-_MAX_INST_WAITS:],
                    on_update=list(si.on_update or []),
                )
            out.append(ins)
        if changed:
            bb.instructions = out


class _PatchedTileContext(tile.TileContext):
    """Split the end-of-kernel drain's sem waits the same way (the drain is
    emitted after scheduling, outside _split_sync_waits' reach)."""

    def _drain_and_barrier(self, tick_clock, wait_clock):
        nc = self.nc
        drain_inst = nc.sync.drain()
        wait_clock.add_sem_waits(
            drain_inst.ins, ScopedClock({None: tick_clock.global_clock})
        )
        ins = drain_inst.ins
        si = ins.sync_info
        waits = list(si.on_wait or []) if si else []
        updates = list(si.on_update or []) if si else []
        if len(waits) > _MAX_DRAIN_WAITS:
            ins.sync_info = mybir.SyncInfo(
                on_wait=waits[:_MAX_DRAIN_WAITS], on_update=updates
            )
            rest = waits[_MAX_DRAIN_WAITS:]
            for i in range(0, len(rest), _MAX_DRAIN_WAITS):
                nop = nc.sync.nop(nofuse=True, hint=f"dw{i}")
                nop.ins.sync_info = mybir.SyncInfo(
                    on_wait=rest[i : i + _MAX_DRAIN_WAITS], on_update=[]
                )
        nc.all_engine_barrier()
        assert self.sems is not None
        popped = nc._tile_sem_poison_stack.pop()
        assert popped is self._sem_poison
        nc.clear_and_free_semaphores(list(self.sems.allocated().values()))
        nc.all_engine_barrier()


def _act_raw(nc, out, in_, func, bias=0.0, scale=1.0):
    """activation() without the Reciprocal guard (table accuracy is plenty for
    1/E at E~2048, and we optionally Newton-polish)."""
    inputs = [nc.scalar.lower_ap(in_)]
    for arg in [bias, scale, 0.0]:
        inputs.append(mybir.ImmediateValue(dtype=mybir.dt.float32, value=arg))
    return nc.scalar.add_instruction(
        mybir.InstActivation(
            name=nc.get_next_instruction_name(),
            func=func,
            ins=inputs,
            outs=[nc.scalar.lower_ap(out)],
        )
    )


def _build(split_waits=True):
    nc = bass.Bass()

    xq_d = nc.dram_tensor("xq", [TQ, 128, 2, CC, 512], FP8, kind="ExternalInput")
    dxq_d = nc.dram_tensor("dxq", [TQ, 128, 2, CC, 512], FP8, kind="ExternalInput")
    bqt_d = nc.dram_tensor("bqt", [128, 2, CC, FQ], FP8, kind="ExternalInput")
    bkt_d = nc.dram_tensor("bkt", [128, 2, CC, DH], FP8, kind="ExternalInput")
    bvt_d = nc.dram_tensor("bvt", [128, 2, CC, DH], FP8, kind="ExternalInput")
    bot_d = nc.dram_tensor("bot", [4, 128, 2, 2, 1024], FP8, kind="ExternalInput")
    sq_d = nc.dram_tensor("sq", [H, DH, 1], F32, kind="ExternalInput")
    sk_d = nc.dram_tensor("sk", [DH, 1], F32, kind="ExternalInput")
    sk16_d = nc.dram_tensor("sk16", [DH, 1], F32, kind="ExternalInput")
    sv_d = nc.dram_tensor("sv", [DH, 1], F32, kind="ExternalInput")
    sv16_d = nc.dram_tensor("sv16", [DH, 1], F32, kind="ExternalInput")
    onescol_d = nc.dram_tensor("onescol", [128, 1], BF16, kind="ExternalInput")
    ones128_d = nc.dram_tensor("ones128", [1, 128], BF16, kind="ExternalInput")
    ident_d = nc.dram_tensor("ident", [128, 128], BF16, kind="ExternalInput")
    y_d = nc.dram_tensor("y", [T, HIDDEN], BF16, kind="ExternalOutput")

    from contextlib import ExitStack
    with _PatchedTileContext(nc) as tc, ExitStack() as _ctx:
        wq = _ctx.enter_context(tc.tile_pool(name="wq", bufs=1))
        wk = _ctx.enter_context(tc.tile_pool(name="wk", bufs=1))
        wv = _ctx.enter_context(tc.tile_pool(name="wv", bufs=1))
        wop = _ctx.enter_context(tc.tile_pool(name="wo", bufs=2))
        xtp = _ctx.enter_context(tc.tile_pool(name="xt", bufs=2))
        dxp = _ctx.enter_context(tc.tile_pool(name="dx", bufs=1))
        qtp = _ctx.enter_context(tc.tile_pool(name="qt", bufs=H))
        ktp = _ctx.enter_context(tc.tile_pool(name="kt", bufs=1))
        vvp = _ctx.enter_context(tc.tile_pool(name="vv", bufs=1))
        vtp = _ctx.enter_context(tc.tile_pool(name="vt", bufs=2))
        ptp = _ctx.enter_context(tc.tile_pool(name="pt", bufs=3))
        pmp = _ctx.enter_context(tc.tile_pool(name="pm", bufs=16))
        rtp = _ctx.enter_context(tc.tile_pool(name="rt", bufs=1))
        omp = _ctx.enter_context(tc.tile_pool(name="om", bufs=2))
        ysp = _ctx.enter_context(tc.tile_pool(name="ys", bufs=3))
        scp = _ctx.enter_context(tc.tile_pool(name="sc", bufs=2))
        t1p = _ctx.enter_context(tc.tile_pool(name="t1", bufs=2))
        esp = _ctx.enter_context(tc.tile_pool(name="es", bufs=4))
        misc = _ctx.enter_context(tc.tile_pool(name="misc", bufs=2))
        rows = _ctx.enter_context(tc.tile_pool(name="rows", bufs=1))
        psM = _ctx.enter_context(tc.tile_pool(name="psM", bufs=2, space="PSUM"))
        psS = _ctx.enter_context(tc.tile_pool(name="psS", bufs=2, space="PSUM"))
        psA = _ctx.enter_context(tc.tile_pool(name="psA", bufs=2, space="PSUM"))
        if True:
            # --- streamed x (fp8) + residual (fp8, x16) -------------------
            xq_sb = {}
            dx_sb = {}

            def load_xq(tq):
                t_ = xtp.tile([128, 2, CC, 512], FP8, tag="xq", name=f"xq{tq}")
                nc.sync.dma_start(t_[:, 0], xq_d[tq, :, 0])
                nc.sync.dma_start(t_[:, 1], xq_d[tq, :, 1])
                xq_sb[tq] = t_

            def load_dx(tq):
                t_ = dxp.tile([128, 2, CC, 512], FP8, tag="dx", name=f"dx{tq}")
                nc.sync.dma_start(t_[:, 0], dxq_d[tq, :, 0])
                nc.sync.dma_start(t_[:, 1], dxq_d[tq, :, 1])
                dx_sb[tq] = t_

            bqt_sb = wq.tile([128, 2, CC, FQ], FP8, tag="wq")
            bkt_sb = wk.tile([128, 2, CC, DH], FP8, tag="wk")
            bvt_sb = wv.tile([128, 2, CC, DH], FP8, tag="wv")
            # interleave bqt pieces with xq(tq0) pieces so Q matmuls start ASAP
            Q4 = CC // 4  # 4 chunks per piece
            xq_sb[0] = xtp.tile([128, 2, CC, 512], FP8, tag="xq", name="xq0")
            for piece in range(4):
                csl = slice(piece * Q4, (piece + 1) * Q4)
                nc.sync.dma_start(bqt_sb[:, 0, csl], bqt_d[:, 0, csl])
                nc.sync.dma_start(bqt_sb[:, 1, csl], bqt_d[:, 1, csl])
                nc.sync.dma_start(xq_sb[0][:, 0, csl], xq_d[0, :, 0, csl])
                nc.sync.dma_start(xq_sb[0][:, 1, csl], xq_d[0, :, 1, csl])
            load_dx(0)
            nc.sync.dma_start(bkt_sb[:, 0], bkt_d[:, 0])
            nc.sync.dma_start(bkt_sb[:, 1], bkt_d[:, 1])
            nc.sync.dma_start(bvt_sb[:, 0], bvt_d[:, 0])
            nc.sync.dma_start(bvt_sb[:, 1], bvt_d[:, 1])

            # --- constants / scales --------------------------------------
            sq_sb = [misc.tile([DH, 1], F32, tag=f"sq{f}", name=f"sq{f}")
                     for f in range(H)]
            for f in range(H):
                nc.sync.dma_start(sq_sb[f][:], sq_d[f])
            sk_sb = misc.tile([DH, 1], F32, tag="sk")
            nc.sync.dma_start(sk_sb[:], sk_d[:])
            sk16_sb = misc.tile([DH, 1], F32, tag="sk16")
            nc.sync.dma_start(sk16_sb[:], sk16_d[:])
            sv_sb = misc.tile([DH, 1], F32, tag="sv")
            nc.sync.dma_start(sv_sb[:], sv_d[:])
            sv16_sb = misc.tile([DH, 1], F32, tag="sv16")
            nc.sync.dma_start(sv16_sb[:], sv16_d[:])
            onescol_sb = misc.tile([128, 1], BF16, tag="onescol")
            nc.sync.dma_start(onescol_sb[:], onescol_d[:])
            ones128_sb = misc.tile([1, 128], BF16, tag="o128")
            nc.sync.dma_start(ones128_sb[:], ones128_d[:])
            ident_sb = misc.tile([128, 128], BF16, tag="ident")
            nc.sync.dma_start(ident_sb[:], ident_d[:])

            # --- persistent activation tiles -----------------------------
            qt_sb = [qtp.tile([DH, T], BF16, tag="qt", name=f"qt{f}")
                     for f in range(H)]
            kt_sb = ktp.tile([DH, T], BF16, tag="kt")
            vv_sb = vvp.tile([128, KT, DH], FP8, tag="vv")
            rt_sb = [rtp.tile([128, 2, T], FP8, tag=f"rt{c}", name=f"rt{c}")
                     for c in range(2)]
            sumv_part = [misc.tile([DH, 1], F32, tag=f"svp{tq}", name=f"svp{tq}")
                         for tq in range(TQ)]

            # --- phase 1: q/k/v projections ------------------------------
            def emit_q(tq, f):
                tsl = slice(tq * 512, (tq + 1) * 512)
                fsl = slice(f * 128, (f + 1) * 128)
                ps = psM.tile([128, 512], F32, tag="mm", name=f"psq{tq}_{f}")
                for cc in range(CC):
                    nc.tensor.matmul(
                        ps[:], bqt_sb[:, :, cc, fsl], xq_sb[tq][:, :, cc, :],
                        start=(cc == 0), stop=(cc == CC - 1), perf_mode=DR,
                    )
                nc.scalar.activation(qt_sb[f][:, tsl], ps[:], AF.Copy,
                                     scale=sq_sb[f][:])

            def emit_kv(tq, which):
                # 2-pass fp8: ps1 = W@xq, ps2 = W@dxq(16x); combine
                # out = ps1*s + ps2*(s/16) in one ACT + one DVE op.
                tsl = slice(tq * 512, (tq + 1) * 512)
                w_sb = bkt_sb if which == "k" else bvt_sb
                s_sb = sk_sb if which == "k" else sv_sb
                s16_sb = sk16_sb if which == "k" else sv16_sb
                ps1 = psM.tile([128, 512], F32, tag="mm", name=f"ps{which}1_{tq}")
                for cc in range(CC):
                    nc.tensor.matmul(
                        ps1[:], w_sb[:, :, cc, :], xq_sb[tq][:, :, cc, :],
                        start=(cc == 0), stop=(cc == CC - 1), perf_mode=DR,
                    )
                ps2 = psA.tile([128, 512], F32, tag="aux", name=f"ps{which}2_{tq}")
                for cc in range(CC):
                    nc.tensor.matmul(
                        ps2[:], w_sb[:, :, cc, :], dx_sb[tq][:, :, cc, :],
                        start=(cc == 0), stop=(cc == CC - 1), perf_mode=DR,
                    )
                a_ = vtp.tile([128, 512], F32, tag="cmb", name=f"a{which}{tq}")
                nc.scalar.activation(a_[:], ps1[:], AF.Copy, scale=s_sb[:])
                if which == "k":
                    nc.vector.scalar_tensor_tensor(
                        out=kt_sb[:, tsl], in0=ps2[:], scalar=s16_sb[:],
                        in1=a_[:], op0=ALU.mult, op1=ALU.add,
                    )
                else:
                    vt_ = vtp.tile([128, 512], BF16, tag="vt", name=f"vt{tq}")
                    nc.vector.scalar_tensor_tensor(
                        out=vt_[:], in0=ps2[:], scalar=s16_sb[:],
                        in1=a_[:], op0=ALU.mult, op1=ALU.add,
                        accum_out=sumv_part[tq][:],
                    )
                    for vt_i in range(4):
                        ps_tr = psS.tile([128, 128], BF16, tag="s2",
                                         name=f"pstr{tq}_{vt_i}")
                        nc.tensor.transpose(
                            ps_tr[:], vt_[:, vt_i * 128:(vt_i + 1) * 128],
                            ident_sb[:],
                        )
                        nc.vector.tensor_copy(
                            out=vv_sb[:, 4 * tq + vt_i, :], in_=ps_tr[:])

            def emit_score_pair(h, qb, kp, ictx):
                # scores pair -> exp (f32) -> Pm1 fp8 cast; bf16 pair pre-sums
                # of the f32 exp tiles feed the E rowsum (errors average out).
                qsl = slice(qb * 512, (qb + 1) * 512)
                ps_s = psS.tile([128, 2, 512], F32, tag="s2",
                                name=f"pss{h}_{qb}_{kp}")
                for j in range(2):
                    kt_i = 2 * kp + j
                    nc.tensor.matmul(
                        ps_s[:, j, :],
                        kt_sb[:, kt_i * 128:(kt_i + 1) * 128],
                        qt_sb[h][:, qsl],
                        start=True, stop=True,
                    )
                pt = ptp.tile([128, 2, 512], F32, tag="pt",
                              name=f"pt{h}_{qb}_{kp}")
                nc.scalar.activation(pt[:], ps_s[:], AF.Exp)
                pm = pmp.tile([128, 2, 512], FP8, tag="pm",
                              name=f"pm{h}_{qb}_{kp}")
                nc.vector.tensor_scalar(
                    out=pm[:], in0=pt[:], scalar1=1.0, scalar2=None,
                    op0=ALU.subtract,
                )
                ictx["pm"].append(pm)
                ictx["pt"].append(pt)
                if kp % 2 == 1:
                    e4 = esp.tile([128, 2, 512], BF16, tag="e4",
                                  name=f"e4_{h}_{qb}_{kp}")
                    nc.vector.tensor_tensor(
                        e4[:], ictx["pt"][kp - 1][:], pt[:], ALU.add)
                    ictx["e4"].append(e4)
                if kp in (3, 7):
                    e2 = esp.tile([128, 2, 512], BF16, tag="e2",
                                  name=f"e2_{h}_{qb}_{kp}")
                    nc.vector.tensor_tensor(
                        e2[:], ictx["e4"][-2][:], ictx["e4"][-1][:], ALU.add)
                    ictx["e2"].append(e2)

            pro_ctx = {"pm": [], "pt": [], "e4": [], "e2": []}
            for tq in range(TQ):
                if tq > 0:
                    load_xq(tq)
                    load_dx(tq)
                if tq < 3:
                    for f in range(H):
                        emit_q(tq, f)
                    emit_kv(tq, "k")
                    emit_kv(tq, "v")
                else:
                    emit_q(3, 0)
                    emit_kv(3, "k")
                    emit_q(3, 1)
                    for kp in range(0, 2):
                        emit_score_pair(0, 0, kp, pro_ctx)
                    emit_q(3, 2)
                    for kp in range(2, 5):
                        emit_score_pair(0, 0, kp, pro_ctx)
                    emit_q(3, 3)
                    for kp in range(5, 8):
                        emit_score_pair(0, 0, kp, pro_ctx)
                    emit_kv(3, "v")

            # --- m preparation (mean attn output, rank-1 terms) ----------
            # sumv = sum_t v[d, t] (sv-scaled);  m = sumv*(256/2048) = sumv/8.
            # m1 = fp8(m); d8 = fp8(m - m1); replicated rows for o_proj inject.
            s01 = misc.tile([DH, 1], F32, tag="s01")
            nc.vector.tensor_tensor(s01[:], sumv_part[0][:], sumv_part[1][:],
                                    ALU.add)
            s23 = misc.tile([DH, 1], F32, tag="s23")
            nc.vector.tensor_tensor(s23[:], sumv_part[2][:], sumv_part[3][:],
                                    ALU.add)
            sumv = misc.tile([DH, 1], F32, tag="sumv")
            nc.vector.tensor_tensor(sumv[:], s01[:], s23[:], ALU.add)
            mb_ = misc.tile([DH, 1], BF16, tag="mb")
            nc.scalar.activation(mb_[:], sumv[:], AF.Copy, scale=0.125)
            ps_mr = psS.tile([1, 128], BF16, tag="s2", name="psmrow")
            nc.tensor.transpose(ps_mr[:], mb_[:], ident_sb[:])
            mrow = misc.tile([1, 128], BF16, tag="mrow")
            nc.vector.tensor_copy(out=mrow[:], in_=ps_mr[:])
            m1row = misc.tile([1, 128], FP8, tag="m1row")
            nc.vector.tensor_copy(out=m1row[:], in_=mrow[:])
            d8row = misc.tile([1, 128], FP8, tag="d8row")
            nc.vector.tensor_tensor(d8row[:], mrow[:], m1row[:], ALU.subtract)
            # replicate m1/d8 along 128 free columns via fp8 outer products
            mrep1 = misc.tile([128, 2, 128], FP8, tag="mrep1")
            mrepd = misc.tile([128, 2, 128], FP8, tag="mrepd")
            for src_row, dst in ((m1row, mrep1), (d8row, mrepd)):
                ps_o1 = psS.tile([128, 128], F32, tag="s2",
                                 name=f"psrep{dst.name}")
                nc.tensor.matmul(ps_o1[:], src_row[:], ones128_sb[:],
                                 start=True, stop=True)
                for i in range(2):
                    nc.scalar.activation(dst[:, i, :], ps_o1[:], AF.Copy)
            negm = misc.tile([DH, 1], F32, tag="negm")
            nc.vector.scalar_tensor_tensor(
                out=negm[:], in0=mrepd[:, 0, 0:1], scalar=-1.0,
                in1=mrep1[:, 0, 0:1], op0=ALU.mult, op1=ALU.subtract,
            )
            # prefetch first o_proj weight slab
            wop_sb = {}

            def load_wop(obp):
                t_ = wop.tile([128, 2, 2, 1024], FP8, tag="wo", name=f"wo{obp}")
                nc.sync.dma_start(t_[:], bot_d[obp])
                wop_sb[obp] = t_

            load_wop(0)

            # --- phase 2: attention, software-pipelined ------------------
            iters = [(h, qb) for qb in range(QB) for h in range(H)]
            pend_sc = []
            pend_fin = []
            prev = (0, 0, pro_ctx)

            def emit_sc(st):
                ph, pqb, hi, lo = st
                ps_sc = psA.tile([128, 512], F32, tag="aux",
                                 name=f"pssc{ph}_{pqb}")
                nc.tensor.matmul(ps_sc[:], ones128_sb[:], hi[:],
                                 start=True, stop=False)
                nc.tensor.matmul(ps_sc[:], ones128_sb[:], lo[:],
                                 start=False, stop=True)
                sc_sb = scp.tile([128, 512], F32, tag="sc",
                                 name=f"sc{ph}_{pqb}")
                nc.scalar.activation(sc_sb[:], ps_sc[:], AF.Copy)
                pend_fin.append((ph, pqb, sc_sb))

            def emit_fin(st, t1_map):
                ph, pqb, sc_sb = st
                qsl = slice(pqb * 512, (pqb + 1) * 512)
                t1 = t1_map.pop((ph, pqb))
                u = t1p.tile([128, 512], F32, tag="u", name=f"u{ph}_{pqb}")
                nc.vector.tensor_tensor(u[:], t1[:], sc_sb[:], ALU.mult)
                c, i = ph // 2, ph % 2
                nc.scalar.activation(rt_sb[c][:, i, qsl], u[:], AF.Identity,
                                     bias=negm[:])

            t1_map = {}
            for idx in range(1, len(iters) + 1):
                cur = iters[idx] if idx < len(iters) else None
                new_ctx = {"pm": [], "pt": [], "e4": [], "e2": []}
                ph, pqb, pctx = prev
                ppm = pctx["pm"]
                ps_o = psM.tile([128, 512], F32, tag="mm",
                                name=f"pso{ph}_{pqb}")
                ps_e = psA.tile([1, 512], F32, tag="aux",
                                name=f"pse{ph}_{pqb}")
                for kpp in range(4):
                    for kp in (2 * kpp, 2 * kpp + 1):
                        if cur is not None:
                            emit_score_pair(cur[0], cur[1], kp, new_ctx)
                    for j in (2 * kpp, 2 * kpp + 1):
                        nc.tensor.matmul(
                            ps_o[:], vv_sb[:, 2 * j:2 * j + 2, :], ppm[j][:],
                            start=(j == 0), stop=(j == KT // 2 - 1),
                            perf_mode=DR,
                        )
                    if kpp >= 2:
                        # E rowsum for prev: 4 bf16 matmuls over the 2
                        # pre-summed [128, 1024] exp tiles
                        t_i = kpp - 2
                        for j in range(2):
                            nc.tensor.matmul(
                                ps_e[:], onescol_sb[:],
                                pctx["e2"][t_i][:, j, :],
                                start=(t_i == 0 and j == 0),
                                stop=(t_i == 1 and j == 1),
                            )
                    if kpp == 1 and pend_sc:
                        emit_sc(pend_sc.pop(0))
                    if kpp == 2 and pend_fin:
                        emit_fin(pend_fin.pop(0), t1_map)
                # E chain for prev: 1/E via ACT table (+2048 bias), Newton
                rcp = rows.tile([1, 512], F32, tag="rcp",
                                name=f"rcp{ph}_{pqb}")
                _act_raw(nc, rcp[:], ps_e[0:1, :], AF.Reciprocal)
                if RECIP_NEWTON:
                    esb = rows.tile([1, 512], F32, tag="esb",
                                    name=f"esb{ph}_{pqb}")
                    nc.scalar.activation(esb[:], ps_e[0:1, :], AF.Copy)
                    # r1 = r0*(2 - E*r0)
                    w_ = rows.tile([1, 512], F32, tag="w_",
                                   name=f"w{ph}_{pqb}")
                    nc.vector.scalar_tensor_tensor(
                        out=w_[:], in0=esb[:], scalar=-1.0, in1=rcp[:],
                        op0=ALU.mult, op1=ALU.mult,
                    )
                    nc.vector.tensor_scalar(
                        out=w_[:], in0=w_[:], scalar1=2.0, scalar2=None,
                        op0=ALU.add,
                    )
                    nc.vector.tensor_tensor(rcp[:], rcp[:], w_[:], ALU.mult)
                hi = rows.tile([1, 512], BF16, tag="hi", name=f"hi{ph}_{pqb}")
                nc.vector.tensor_scalar(out=hi[:], in0=rcp[:], scalar1=256.0,
                                        scalar2=None, op0=ALU.mult)
                lo = rows.tile([1, 512], BF16, tag="lo", name=f"lo{ph}_{pqb}")
                nc.vector.scalar_tensor_tensor(
                    out=lo[:], in0=rcp[:], scalar=256.0, in1=hi[:],
                    op0=ALU.mult, op1=ALU.subtract,
                )
                t1 = t1p.tile([128, 512], F32, tag="t1", name=f"t1{ph}_{pqb}")
                nc.scalar.activation(t1[:], ps_o[:], AF.Identity,
                                     bias=sumv[:])
                t1_map[(ph, pqb)] = t1
                pend_sc.append((ph, pqb, hi, lo))
                prev = (cur[0], cur[1], new_ctx) if cur else None

            # --- phase 3: o_proj partial (x256, r-centered) --------------
            for obp in range(4):
                if obp < 3:
                    load_wop(obp + 1)
                w_t = wop_sb[obp]
                ps_om = psS.tile([128, 1024], F32, tag="s2",
                                 name=f"psom{obp}")
                for jo in range(2):
                    osl = slice(jo * 512, (jo + 1) * 512)
                    k_ = 0
                    for mrep in (mrep1, mrepd):
                        for c in range(2):
                            nc.tensor.matmul(
                                ps_om[:, osl], mrep[:], w_t[:, c, :, osl],
                                start=(k_ == 0), stop=(k_ == 3), perf_mode=DR,
                            )
                            k_ += 1
                om_sb = omp.tile([128, 1024], F32, tag="om",
                                 name=f"om{obp}")
                nc.scalar.activation(om_sb[:], ps_om[:], AF.Copy)
                for tt in range(16):
                    tsl = slice(tt * 128, (tt + 1) * 128)
                    ps_y = psS.tile([128, 1024], F32, tag="s2",
                                    name=f"psy{obp}_{tt}")
                    for jo in range(2):
                        osl = slice(jo * 512, (jo + 1) * 512)
                        for c in range(2):
                            nc.tensor.matmul(
                                ps_y[:, osl], rt_sb[c][:, :, tsl],
                                w_t[:, c, :, osl],
                                start=(c == 0), stop=(c == 1), perf_mode=DR,
                            )
                    if obp == 0 and tt == 2 and pend_sc:
                        emit_sc(pend_sc.pop(0))
                    if obp == 0 and tt == 6 and pend_fin:
                        emit_fin(pend_fin.pop(0), t1_map)
                    ysb = ysp.tile([128, 1024], BF16, tag="ys",
                                   name=f"ys{obp}_{tt}")
                    nc.vector.tensor_tensor(ysb[:], ps_y[:], om_sb[:], ALU.add)
                    nc.sync.dma_start(
                        y_d[tt * 128:(tt + 1) * 128,
                            obp * 1024:(obp + 1) * 1024], ysb[:]
                    )

    if split_waits:
        _split_sync_waits(nc)
    return nc


_NC_CACHE = None


def _get_nc():
    global _NC_CACHE
    if _NC_CACHE is None:
        _NC_CACHE = _build()
    return _NC_CACHE


def _binarize(w):
    """Match reference bitnet_linear: s = max(mean|W|_row, 1e-8) (>0), so
    sign(W/s) == sign(W). Returns (sign(W) as fp8, s as f32)."""
    w = np.asarray(w, np.float32)
    s = np.maximum(
        np.abs(w).mean(axis=1, dtype=np.float64).astype(np.float32), 1e-8
    )
    return np.sign(w).astype(NPF8), s


def _pack_w(wt, nf):
    # [4096 in, nf out] -> [128, 2, CC, nf]; in-feature f = cc*256 + i*128 + p
    return np.ascontiguousarray(
        wt.reshape(CC, 2, 128, nf).transpose(2, 1, 0, 3))


def _make_in_maps(hidden_states, q_weight, q_scale, k_weight, k_scale,
                  v_weight, v_scale, o_weight, o_scale):
    hs = np.asarray(hidden_states, np.float32)
    b, t, hid = hs.shape
    assert (b, t, hid) == (1, T, HIDDEN)

    xT = np.ascontiguousarray(hs[0].T)                  # [4096, T] f32
    xq8 = xT.astype(NPF8)
    dx8 = ((xT - xq8.astype(np.float32)) * np.float32(16.0)).astype(NPF8)

    def pack_x(a):
        # [4096, T] -> [TQ, 128, 2, CC, 512]
        return np.ascontiguousarray(
            a.reshape(CC, 2, 128, TQ, 512).transpose(3, 2, 1, 0, 4))

    xq4 = pack_x(xq8)
    dx4 = pack_x(dx8)

    bq, s_q = _binarize(q_weight)
    bk, s_k = _binarize(k_weight)
    bv, s_v = _binarize(v_weight)
    bo, s_o = _binarize(o_weight)

    sq_full = s_q * np.asarray(q_scale, np.float32)                # [4096]
    sk_full = s_k * np.asarray(k_scale, np.float32) / np.float32(
        np.sqrt(DH))                                               # [1024]
    sv_full = s_v * np.asarray(v_scale, np.float32)                # [1024]
    so_full = s_o * np.asarray(o_scale, np.float32)                # [4096]

    onescol = np.ones((128, 1), NPBF)
    ones128 = np.ones((1, 128), NPBF)
    ident = np.eye(128, dtype=NPBF)

    in_maps = []
    for i in range(N_CORES):
        fq = slice(FQ * i, FQ * (i + 1))
        fk = slice(DH * i, DH * (i + 1))
        # o weights: [4096 o, 512 f] -> [4 obp, 128 p, 2 c, 2 i, 1024 o]
        bo_t = np.ascontiguousarray(bo[:, fq].T)        # [512 f, 4096 o] fp8
        bo_p = np.ascontiguousarray(
            bo_t.reshape(2, 2, 128, 4, 1024).transpose(3, 2, 0, 1, 4))
        sk_c = sk_full[fk].reshape(DH, 1)
        sv_c = sv_full[fk].reshape(DH, 1)
        in_maps.append({
            "xq": xq4,
            "dxq": dx4,
            "bqt": _pack_w(np.ascontiguousarray(bq[fq].T), FQ),
            "bkt": _pack_w(np.ascontiguousarray(bk[fk].T), DH),
            "bvt": _pack_w(np.ascontiguousarray(bv[fk].T), DH),
            "bot": bo_p,
            "sq": np.ascontiguousarray(
                sq_full[fq].reshape(H, DH, 1).astype(np.float32)),
            "sk": np.ascontiguousarray(sk_c.astype(np.float32)),
            "sk16": np.ascontiguousarray((sk_c / 16.0).astype(np.float32)),
            "sv": np.ascontiguousarray(sv_c.astype(np.float32)),
            "sv16": np.ascontiguousarray((sv_c / 16.0).astype(np.float32)),
            "onescol": onescol,
            "ones128": ones128,
            "ident": ident,
        })
    return in_maps, so_full


def kernel(**inputs):
    in_maps, so_full = _make_in_maps(**inputs)
    nc = _get_nc()
    res = run_bass_kernel_spmd(
        nc, in_maps, core_ids=list(range(N_CORES)), trace=TRACE,
        trace_cores=list(range(N_CORES)) if TRACE and TRACE_ALL_CORES else None,
    )
    if TRACE:
        kernel.last_exec_time_ns = res.exec_time_ns
        kernel.last_mean_exec_time_ns = res.mean_exec_time_ns

    y = np.zeros((T, HIDDEN), np.float32)
    for i in range(N_CORES):
        y += res.results[i]["y"].astype(np.float32)
    y *= so_full[None, :] * np.float32(1.0 / 256.0)
    return y.reshape(1, T, HIDDEN)


# revision 22
# speedup vs baseline: 1.0698x; 1.0698x over previous
"""BitNet attention (GQA, 32 q-heads / 8 kv-heads, hidden 4096, seq 2048) on 8
Trainium2 NeuronCores — fp8 DoubleRow edition.

Sharding: tensor-parallel over heads (core i: q-heads 4i..4i+3, kv-head i,
o_proj contribution of its 512 hidden columns; host sums the 8 partials).

Main idea vs the bf16 version: BitNet weights are exactly +-1, which fp8-e4m3
represents exactly, and the TRN2 PE runs fp8 matmuls in DoubleRow perf mode at
2x bf16 throughput (K=256 per instruction).  Quantization noise is held inside
the rel-err budget by exploiting the structure of the problem:
  - scores are O(0.25) and the softmax nearly uniform, so fp8 noise on x
    entering Q/K (weights exact) is strongly suppressed through the softmax;
    K additionally runs a 2-pass (x + residual/16) split which is free vs
    bf16 (K proj is 8x smaller than Q).
  - V runs the 2-pass split too (full bf16-level precision at fp8 speed).
  - the attention-value product uses *centered* probabilities:  P@v =
    sum(v) + (P-1)@v.  P-1 is O(0.25) so its fp8 quantization error is ~4x
    smaller than quantizing P~1 directly; the exact rank-1 sum(v) term is
    restored via a per-partition ACT bias.
  - o_proj is centered the same way: r = attn*256/E - m (m = mean attention
    output, a per-d constant) is quantized fp8; the dominant rank-1 term
    W_o @ m is added back exactly via replicated-m DoubleRow matmuls
    (m split into fp8 m1 + residual, both exact on the PE).
  - softmax denominator E = 2048 + rowsum(P-1) comes from DoubleRow rowsums
    with a [128,2,4]-ones stationary; 1/E from the ACT Reciprocal table
    (bias=+2048 fused), optionally polished by one DVE Newton step.
  - the per-query 1/E scale is applied through a rank-1 outer product
    materialized from a bf16 hi/lo split (f32-accurate), as PE work.
Outputs are scaled by 256 on-device (fp8 subnormal avoidance for r); the host
divides by 256 when applying the final o-scales.
"""

import numpy as np
import ml_dtypes

import concourse.bass as bass
import concourse.mybir as mybir
import concourse.tile as tile
from concourse.vector_clock import ScopedClock
from concourse.bass_utils import run_bass_kernel_spmd

F32 = mybir.dt.float32
BF16 = mybir.dt.bfloat16
FP8 = mybir.dt.float8e4
NPF8 = ml_dtypes.float8_e4m3
NPBF = ml_dtypes.bfloat16

HIDDEN = 4096
T = 2048
N_CORES = 8
FQ = HIDDEN // N_CORES   # 512 q-features per core
H = 4                    # q heads per core
DH = 128                 # head dim
CC = 16                  # 256-wide contraction chunks (fp8 DoubleRow)
TQ = 4                   # token quarters (512 tokens each)
KT = 16                  # key tiles of 128
QB = 4                   # query blocks of 512
ALU = mybir.AluOpType
AF = mybir.ActivationFunctionType
DR = mybir.MatmulPerfMode.DoubleRow

TRACE = False            # set by test.py for profiling runs
TRACE_ALL_CORES = False

RECIP_NEWTON = False     # ACT-table 1/E measured at 1.16e-5 rel err

_MAX_DRAIN_WAITS = 1
_MAX_INST_WAITS = 1


def _split_sync_waits(nc):
    """The walrus build in this container rejects instructions carrying more
    than one sync wait ("Too many sync wait commands"). Cap every instruction
    at _MAX_INST_WAITS waits; spill the excess onto InstEventSemaphore
    (standalone wait) instructions inserted immediately before on the same
    engine (engines are in-order, so combined wait semantics are identical)."""
    counter = [0]

    def _mk_wait(engine, waits):
        counter[0] += 1
        nop = mybir.InstEventSemaphore(
            name=f"waitsplit_{counter[0]}", ins=[], outs=[]
        )
        nop.engine = engine
        nop.sync_info = mybir.SyncInfo(on_wait=list(waits), on_update=[])
        nc.register_instruction(nop, overwrite=True)
        return nop

    for bb in nc.main_func.blocks:
        insts = list(bb.instructions)
        out = []
        changed = False
        for ins in insts:
            si = ins.sync_info
            waits = list(si.on_wait or []) if si else []
            if len(waits) > _MAX_INST_WAITS:
                changed = True
                rest = waits[:-_MAX_INST_WAITS]
                for i in range(0, len(rest), _MAX_INST_WAITS):
                    out.append(_mk_wait(ins.engine, rest[i : i + _MAX_INST_WAITS]))
                ins.sync_info = mybir.SyncInfo(
                    on_wait=waits[-_MAX_INST_WAITS:],
                    on_update=list(si.on_update or []),
                )
            out.append(ins)
        if changed:
            bb.instructions = out


class _PatchedTileContext(tile.TileContext):
    """Split the end-of-kernel drain's sem waits the same way (the drain is
    emitted after scheduling, outside _split_sync_waits' reach)."""

    def _drain_and_barrier(self, tick_clock, wait_clock):
        nc = self.nc
        drain_inst = nc.sync.drain()
        wait_clock.add_sem_waits(
            drain_inst.ins, ScopedClock({None: tick_clock.global_clock})
        )
        ins = drain_inst.ins
        si = ins.sync_info
        waits = list(si.on_wait or []) if si else []
        updates = list(si.on_update or []) if si else []
        if len(waits) > _MAX_DRAIN_WAITS:
            ins.sync_info = mybir.SyncInfo(
                on_wait=waits[:_MAX_DRAIN_WAITS], on_update=updates
            )
            rest = waits[_MAX_DRAIN_WAITS:]
            for i in range(0, len(rest), _MAX_DRAIN_WAITS):
                nop = nc.sync.nop(nofuse=True, hint=f"dw{i}")
                nop.ins.sync_info = mybir.SyncInfo(
                    on_wait=rest[i : i + _MAX_DRAIN_WAITS], on_update=[]
                )
        nc.all_engine_barrier()
        assert self.sems is not None
        popped = nc._tile_sem_poison_stack.pop()
        assert popped is self._sem_poison
        nc.clear_and_free_semaphores(list(self.sems.allocated().values()))
        nc.all_engine_barrier()


def _act_raw(nc, out, in_, func, bias=0.0, scale=1.0):
    """activation() without the Reciprocal guard (table accuracy is plenty for
    1/E at E~2048, and we optionally Newton-polish)."""
    inputs = [nc.scalar.lower_ap(in_)]
    for arg in [bias, scale, 0.0]:
        inputs.append(mybir.ImmediateValue(dtype=mybir.dt.float32, value=arg))
    return nc.scalar.add_instruction(
        mybir.InstActivation(
            name=nc.get_next_instruction_name(),
            func=func,
            ins=inputs,
            outs=[nc.scalar.lower_ap(out)],
        )
    )


def _build(split_waits=True):
    nc = bass.Bass()

    xq_d = nc.dram_tensor("xq", [TQ, 128, 2, CC, 512], FP8, kind="ExternalInput")
    dxq_d = nc.dram_tensor("dxq", [TQ, 128, 2, CC, 512], FP8, kind="ExternalInput")
    bqt_d = nc.dram_tensor("bqt", [128, 2, CC, FQ], FP8, kind="ExternalInput")
    bkt_d = nc.dram_tensor("bkt", [128, 2, CC, DH], FP8, kind="ExternalInput")
    bvt_d = nc.dram_tensor("bvt", [128, 2, CC, DH], FP8, kind="ExternalInput")
    bot_d = nc.dram_tensor("bot", [4, 128, 2, 2, 1024], FP8, kind="ExternalInput")
    sq_d = nc.dram_tensor("sq", [H, DH, 1], F32, kind="ExternalInput")
    sk_d = nc.dram_tensor("sk", [DH, 1], F32, kind="ExternalInput")
    sv_d = nc.dram_tensor("sv", [DH, 1], F32, kind="ExternalInput")
    ones8_d = nc.dram_tensor("ones8", [128, 2, DH], FP8, kind="ExternalInput")
    ones128_d = nc.dram_tensor("ones128", [1, 128], BF16, kind="ExternalInput")
    ident_d = nc.dram_tensor("ident", [128, 128], BF16, kind="ExternalInput")
    y_d = nc.dram_tensor("y", [T, HIDDEN], BF16, kind="ExternalOutput")
    om_d = nc.dram_tensor("om", [4, 1024], BF16, kind="ExternalOutput")

    from contextlib import ExitStack
    with _PatchedTileContext(nc) as tc, ExitStack() as _ctx:
        wq = _ctx.enter_context(tc.tile_pool(name="wq", bufs=1))
        wk = _ctx.enter_context(tc.tile_pool(name="wk", bufs=1))
        wv = _ctx.enter_context(tc.tile_pool(name="wv", bufs=1))
        wop = _ctx.enter_context(tc.tile_pool(name="wo", bufs=1))
        xtp = _ctx.enter_context(tc.tile_pool(name="xt", bufs=2))
        dxp = _ctx.enter_context(tc.tile_pool(name="dx", bufs=1))
        qtp = _ctx.enter_context(tc.tile_pool(name="qt", bufs=H))
        ktp = _ctx.enter_context(tc.tile_pool(name="kt", bufs=1))
        vvp = _ctx.enter_context(tc.tile_pool(name="vv", bufs=1))
        vtp = _ctx.enter_context(tc.tile_pool(name="vt", bufs=2))
        ptp = _ctx.enter_context(tc.tile_pool(name="pt", bufs=2))
        pmp = _ctx.enter_context(tc.tile_pool(name="pm", bufs=16))
        rtp = _ctx.enter_context(tc.tile_pool(name="rt", bufs=1))
        omp = _ctx.enter_context(tc.tile_pool(name="om", bufs=1))
        ysp = _ctx.enter_context(tc.tile_pool(name="ys", bufs=3))
        scp = _ctx.enter_context(tc.tile_pool(name="sc", bufs=2))
        t1p = _ctx.enter_context(tc.tile_pool(name="t1", bufs=6))
        up = _ctx.enter_context(tc.tile_pool(name="up", bufs=2))
        rows4 = _ctx.enter_context(tc.tile_pool(name="rows4", bufs=4))
        misc = _ctx.enter_context(tc.tile_pool(name="misc", bufs=2))
        rows = _ctx.enter_context(tc.tile_pool(name="rows", bufs=1))
        psM = _ctx.enter_context(tc.tile_pool(name="psM", bufs=2, space="PSUM"))
        psS = _ctx.enter_context(tc.tile_pool(name="psS", bufs=2, space="PSUM"))
        psA = _ctx.enter_context(tc.tile_pool(name="psA", bufs=2, space="PSUM"))
        if True:
            # --- streamed x (fp8) + residual (fp8, x16) -------------------
            xq_sb = {}
            dx_sb = {}

            def load_xq(tq):
                t_ = xtp.tile([128, 2, CC, 512], FP8, tag="xq", name=f"xq{tq}")
                nc.sync.dma_start(t_[:, 0], xq_d[tq, :, 0])
                nc.sync.dma_start(t_[:, 1], xq_d[tq, :, 1])
                xq_sb[tq] = t_

            def load_dx(tq):
                t_ = dxp.tile([128, 2, CC, 512], FP8, tag="dx", name=f"dx{tq}")
                nc.sync.dma_start(t_[:, 0], dxq_d[tq, :, 0])
                nc.sync.dma_start(t_[:, 1], dxq_d[tq, :, 1])
                dx_sb[tq] = t_

            bqt_sb = wq.tile([128, 2, CC, FQ], FP8, tag="wq")
            bkt_sb = wk.tile([128, 2, CC, DH], FP8, tag="wk")
            bvt_sb = wv.tile([128, 2, CC, DH], FP8, tag="wv")
            # interleave bqt pieces with xq(tq0) pieces so Q matmuls start ASAP
            Q4 = CC // 4  # 4 chunks per piece
            xq_sb[0] = xtp.tile([128, 2, CC, 512], FP8, tag="xq", name="xq0")
            for piece in range(4):
                csl = slice(piece * Q4, (piece + 1) * Q4)
                nc.sync.dma_start(bqt_sb[:, 0, csl], bqt_d[:, 0, csl])
                nc.sync.dma_start(bqt_sb[:, 1, csl], bqt_d[:, 1, csl])
                nc.sync.dma_start(xq_sb[0][:, 0, csl], xq_d[0, :, 0, csl])
                nc.sync.dma_start(xq_sb[0][:, 1, csl], xq_d[0, :, 1, csl])
            load_dx(0)
            nc.sync.dma_start(bkt_sb[:, 0], bkt_d[:, 0])
            nc.sync.dma_start(bkt_sb[:, 1], bkt_d[:, 1])
            nc.sync.dma_start(bvt_sb[:, 0], bvt_d[:, 0])
            nc.sync.dma_start(bvt_sb[:, 1], bvt_d[:, 1])

            # --- constants / scales --------------------------------------
            sq_sb = [misc.tile([DH, 1], F32, tag=f"sq{f}", name=f"sq{f}")
                     for f in range(H)]
            for f in range(H):
                nc.sync.dma_start(sq_sb[f][:], sq_d[f])
            sk_sb = misc.tile([DH, 1], F32, tag="sk")
            nc.sync.dma_start(sk_sb[:], sk_d[:])
            sv_sb = misc.tile([DH, 1], F32, tag="sv")
            nc.sync.dma_start(sv_sb[:], sv_d[:])
            ones8_sb = misc.tile([128, 2, DH], FP8, tag="ones8")
            nc.sync.dma_start(ones8_sb[:], ones8_d[:])
            ones128_sb = misc.tile([1, 128], BF16, tag="o128")
            nc.sync.dma_start(ones128_sb[:], ones128_d[:])
            ident_sb = misc.tile([128, 128], BF16, tag="ident")
            nc.sync.dma_start(ident_sb[:], ident_d[:])

            # --- persistent activation tiles -----------------------------
            qt_sb = [qtp.tile([DH, T], BF16, tag="qt", name=f"qt{f}")
                     for f in range(H)]
            kt_sb = ktp.tile([DH, T], BF16, tag="kt")
            vv_sb = vvp.tile([128, KT, DH], FP8, tag="vv")
            rt_sb = [rtp.tile([128, 2, T], FP8, tag=f"rt{c}", name=f"rt{c}")
                     for c in range(2)]
            sumv_part = [misc.tile([DH, 1], F32, tag=f"svp{tq}", name=f"svp{tq}")
                         for tq in range(TQ)]

            # --- phase 1: q/k/v projections ------------------------------
            def emit_q(tq, f):
                tsl = slice(tq * 512, (tq + 1) * 512)
                fsl = slice(f * 128, (f + 1) * 128)
                ps = psM.tile([128, 512], F32, tag="mm", name=f"psq{tq}_{f}")
                for cc in range(CC):
                    nc.tensor.matmul(
                        ps[:], bqt_sb[:, :, cc, fsl], xq_sb[tq][:, :, cc, :],
                        start=(cc == 0), stop=(cc == CC - 1), perf_mode=DR,
                    )
                nc.scalar.activation(qt_sb[f][:, tsl], ps[:], AF.Copy,
                                     scale=sq_sb[f][:])

            def emit_kv(tq, which):
                # one 32-chunk fp8 accumulation: W@xq + W@dx (dx = unscaled
                # fp8 residual of x; subnormal absolute error ~2^-10 is fine)
                tsl = slice(tq * 512, (tq + 1) * 512)
                w_sb = bkt_sb if which == "k" else bvt_sb
                s_sb = sk_sb if which == "k" else sv_sb
                ps1 = psM.tile([128, 512], F32, tag="mm", name=f"ps{which}1_{tq}")
                for cc in range(CC):
                    nc.tensor.matmul(
                        ps1[:], w_sb[:, :, cc, :], xq_sb[tq][:, :, cc, :],
                        start=(cc == 0), stop=False, perf_mode=DR,
                    )
                for cc in range(CC):
                    nc.tensor.matmul(
                        ps1[:], w_sb[:, :, cc, :], dx_sb[tq][:, :, cc, :],
                        start=False, stop=(cc == CC - 1), perf_mode=DR,
                    )
                if which == "k":
                    nc.scalar.activation(kt_sb[:, tsl], ps1[:], AF.Copy,
                                         scale=sk_sb[:])
                else:
                    vt_ = vtp.tile([128, 512], BF16, tag="vt", name=f"vt{tq}")
                    nc.scalar.activation(vt_[:], ps1[:], AF.Copy,
                                         scale=sv_sb[:],
                                         accum_out=sumv_part[tq][:])
                    for vt_i in range(4):
                        ps_tr = psS.tile([128, 128], BF16, tag="s2",
                                         name=f"pstr{tq}_{vt_i}")
                        nc.tensor.transpose(
                            ps_tr[:], vt_[:, vt_i * 128:(vt_i + 1) * 128],
                            ident_sb[:],
                        )
                        nc.vector.tensor_copy(
                            out=vv_sb[:, 4 * tq + vt_i, :], in_=ps_tr[:])

            def emit_score_pair(h, qb, kp, ictx):
                # scores pair -> exp (f32) -> Pm1 fp8 cast
                qsl = slice(qb * 512, (qb + 1) * 512)
                ps_s = psS.tile([128, 2, 512], F32, tag="s2",
                                name=f"pss{h}_{qb}_{kp}")
                for j in range(2):
                    kt_i = 2 * kp + j
                    nc.tensor.matmul(
                        ps_s[:, j, :],
                        kt_sb[:, kt_i * 128:(kt_i + 1) * 128],
                        qt_sb[h][:, qsl],
                        start=True, stop=True,
                    )
                pt = ptp.tile([128, 2, 512], F32, tag="pt",
                              name=f"pt{h}_{qb}_{kp}")
                nc.scalar.activation(pt[:], ps_s[:], AF.Exp)
                pm = pmp.tile([128, 2, 512], FP8, tag="pm",
                              name=f"pm{h}_{qb}_{kp}")
                nc.vector.tensor_scalar(
                    out=pm[:], in0=pt[:], scalar1=1.0, scalar2=None,
                    op0=ALU.subtract,
                )
                ictx["pm"].append(pm)

            pro_ctx = {"pm": []}
            for tq in range(TQ):
                if tq < 3:
                    emit_q(tq, 0)
                    emit_q(tq, 1)
                    if tq < TQ - 1:
                        load_xq(tq + 1)
                        load_dx(tq + 1)
                    emit_kv(tq, "k")
                    emit_q(tq, 2)
                    emit_q(tq, 3)
                    emit_kv(tq, "v")
                else:
                    emit_kv(3, "k")
                    emit_q(3, 0)
                    emit_q(3, 1)
                    for kp in range(0, 2):
                        emit_score_pair(0, 0, kp, pro_ctx)
                    emit_q(3, 2)
                    for kp in range(2, 5):
                        emit_score_pair(0, 0, kp, pro_ctx)
                    emit_q(3, 3)
                    for kp in range(5, 8):
                        emit_score_pair(0, 0, kp, pro_ctx)
                    emit_kv(3, "v")

            # --- m preparation (mean attn output, rank-1 terms) ----------
            # sumv = sum_t v[d, t] (sv-scaled);  m = sumv*(256/2048) = sumv/8.
            # m1 = fp8(m); d8 = fp8(m - m1); replicated rows for o_proj inject.
            s01 = misc.tile([DH, 1], F32, tag="s01")
            nc.vector.tensor_tensor(s01[:], sumv_part[0][:], sumv_part[1][:],
                                    ALU.add)
            s23 = misc.tile([DH, 1], F32, tag="s23")
            nc.vector.tensor_tensor(s23[:], sumv_part[2][:], sumv_part[3][:],
                                    ALU.add)
            sumv = misc.tile([DH, 1], F32, tag="sumv")
            nc.vector.tensor_tensor(sumv[:], s01[:], s23[:], ALU.add)
            mb_ = misc.tile([DH, 1], BF16, tag="mb")
            nc.scalar.activation(mb_[:], sumv[:], AF.Copy, scale=0.125)
            ps_mr = psS.tile([1, 128], BF16, tag="s2", name="psmrow")
            nc.tensor.transpose(ps_mr[:], mb_[:], ident_sb[:])
            mrow = misc.tile([1, 128], BF16, tag="mrow")
            nc.vector.tensor_copy(out=mrow[:], in_=ps_mr[:])
            m1row = misc.tile([1, 128], FP8, tag="m1row")
            nc.vector.tensor_copy(out=m1row[:], in_=mrow[:])
            d8row = misc.tile([1, 128], FP8, tag="d8row")
            nc.vector.tensor_tensor(d8row[:], mrow[:], m1row[:], ALU.subtract)
            # replicate m1/d8 along 128 free columns via fp8 outer products
            mrep1 = misc.tile([128, 2, 128], FP8, tag="mrep1")
            mrepd = misc.tile([128, 2, 128], FP8, tag="mrepd")
            for src_row, dst in ((m1row, mrep1), (d8row, mrepd)):
                ps_o1 = psS.tile([128, 128], F32, tag="s2",
                                 name=f"psrep{dst.name}")
                nc.tensor.matmul(ps_o1[:], src_row[:], ones128_sb[:],
                                 start=True, stop=True)
                for i in range(2):
                    nc.scalar.activation(dst[:, i, :], ps_o1[:], AF.Copy)
            negm = misc.tile([DH, 1], F32, tag="negm")
            nc.vector.scalar_tensor_tensor(
                out=negm[:], in0=mrepd[:, 0, 0:1], scalar=-1.0,
                in1=mrep1[:, 0, 0:1], op0=ALU.mult, op1=ALU.subtract,
            )
            # o_proj weights resident; om tiles computed upfront
            wop_sb = {}

            def load_wop(obp):
                t_ = wop.tile([128, 2, 2, 1024], FP8, tag=f"wo{obp}",
                              name=f"wo{obp}")
                nc.sync.dma_start(t_[:], bot_d[obp])
                wop_sb[obp] = t_

            for obp in range(4):
                load_wop(obp)
            om_sb = {}

            def emit_om(obp):
                w_t = wop_sb[obp]
                ps_om = psS.tile([128, 1024], F32, tag="s2",
                                 name=f"psom{obp}")
                for jo in range(2):
                    osl = slice(jo * 512, (jo + 1) * 512)
                    k_ = 0
                    for mrep in (mrep1, mrepd):
                        for c in range(2):
                            nc.tensor.matmul(
                                ps_om[:, osl], mrep[:], w_t[:, c, :, osl],
                                start=(k_ == 0), stop=(k_ == 3), perf_mode=DR,
                            )
                            k_ += 1
                t_ = omp.tile([128, 1024], BF16, tag=f"om{obp}",
                              name=f"om{obp}")
                nc.scalar.activation(t_[:], ps_om[:], AF.Copy)
                nc.sync.dma_start(om_d[obp:obp + 1, :], t_[0:1, :])
                om_sb[obp] = t_

            def emit_oproj_obp_tt(obp, tt):
                tsl = slice(tt * 128, (tt + 1) * 128)
                w_t = wop_sb[obp]
                ps_y = psS.tile([128, 1024], F32, tag="s2",
                                name=f"psy{obp}_{tt}")
                for jo in range(2):
                    osl = slice(jo * 512, (jo + 1) * 512)
                    for c in range(2):
                        nc.tensor.matmul(
                            ps_y[:, osl], rt_sb[c][:, :, tsl],
                            w_t[:, c, :, osl],
                            start=(c == 0), stop=(c == 1), perf_mode=DR,
                        )
                ysb = ysp.tile([128, 1024], BF16, tag="ys",
                               name=f"ys{obp}_{tt}")
                nc.vector.tensor_copy(out=ysb[:], in_=ps_y[:])
                nc.sync.dma_start(
                    y_d[tsl, obp * 1024:(obp + 1) * 1024], ysb[:]
                )

            # --- phase 2: attention, software-pipelined ------------------
            iters = [(h, qb) for qb in range(QB) for h in range(H)]
            pend_sc = []
            pend_fin = []
            prev = (0, 0, pro_ctx)

            def emit_sc(st):
                # hi/lo rows were computed at recip time; just the outers here
                ph, pqb, hi, lo = st
                ps_sc = psA.tile([128, 512], F32, tag="aux",
                                 name=f"pssc{ph}_{pqb}")
                nc.tensor.matmul(ps_sc[:], ones128_sb[:], hi[:],
                                 start=True, stop=False)
                nc.tensor.matmul(ps_sc[:], ones128_sb[:], lo[:],
                                 start=False, stop=True)
                sc_sb = scp.tile([128, 512], F32, tag="sc",
                                 name=f"sc{ph}_{pqb}")
                nc.scalar.activation(sc_sb[:], ps_sc[:], AF.Copy)
                pend_fin.append((ph, pqb, sc_sb))

            def emit_fin(st, t1_map):
                ph, pqb, sc_sb = st
                qsl = slice(pqb * 512, (pqb + 1) * 512)
                t1 = t1_map.pop((ph, pqb))
                u = up.tile([128, 512], F32, tag="u", name=f"u{ph}_{pqb}")
                nc.vector.tensor_tensor(u[:], t1[:], sc_sb[:], ALU.mult)
                c, i = ph // 2, ph % 2
                nc.vector.tensor_scalar(
                    out=rt_sb[c][:, i, qsl], in0=u[:], scalar1=negm[:],
                    scalar2=None, op0=ALU.add,
                )

            t1_map = {}
            e_map = {}
            pend_tt = []
            fin_done = [0] * QB
            for idx in range(1, len(iters) + 1):
                cur = iters[idx] if idx < len(iters) else None
                new_ctx = {"pm": []}
                ph, pqb, pctx = prev
                ppm = pctx["pm"]
                ps_o = psM.tile([128, 512], F32, tag="mm",
                                name=f"pso{ph}_{pqb}")
                ps_e = psA.tile([128, 512], F32, tag="aux",
                                name=f"pse{ph}_{pqb}")
                for kpp in range(4):
                    if cur is not None:
                        emit_score_pair(cur[0], cur[1], 2 * kpp, new_ctx)
                    nc.tensor.matmul(
                        ps_o[:], vv_sb[:, 4 * kpp:4 * kpp + 2, :],
                        ppm[2 * kpp][:],
                        start=(kpp == 0), stop=False, perf_mode=DR,
                    )
                    nc.tensor.matmul(
                        ps_e[:], ones8_sb[:], ppm[2 * kpp][:],
                        start=(kpp == 0), stop=False, perf_mode=DR,
                    )
                    if cur is not None:
                        emit_score_pair(cur[0], cur[1], 2 * kpp + 1, new_ctx)
                    nc.tensor.matmul(
                        ps_o[:], vv_sb[:, 4 * kpp + 2:4 * kpp + 4, :],
                        ppm[2 * kpp + 1][:],
                        start=False, stop=(kpp == 3), perf_mode=DR,
                    )
                    nc.tensor.matmul(
                        ps_e[:], ones8_sb[:], ppm[2 * kpp + 1][:],
                        start=False, stop=(kpp == 3), perf_mode=DR,
                    )
                    if kpp == 2 and pend_sc:
                        emit_sc(pend_sc.pop(0))
                    if kpp == 3 and pend_fin:
                        emit_fin(pend_fin.pop(0), t1_map)
                e_sb = rows4.tile([1, 512], F32, tag="esb",
                                 name=f"esb{ph}_{pqb}")
                nc.vector.tensor_copy(out=e_sb[:], in_=ps_e[0:1, :])
                e_map[(ph, pqb)] = e_sb
                t1 = t1p.tile([128, 512], F32, tag="t1", name=f"t1{ph}_{pqb}")
                nc.scalar.activation(t1[:], ps_o[:], AF.Identity,
                                     bias=sumv[:])
                t1_map[(ph, pqb)] = t1
                if ph == H - 1:
                    # qb complete: 4 table-batched reciprocals at body end
                    # (between two exp runs: 2 table loads per qb)
                    # rcp = 1/(e/256 + 8) = 256/(e+2048) = 256/E
                    for h2 in range(H):
                        rcp = rows.tile([1, 512], F32, tag="rcp",
                                        name=f"rcp{h2}_{pqb}")
                        _act_raw(nc, rcp[:], e_map.pop((h2, pqb))[:],
                                 AF.Reciprocal, bias=8.0, scale=1.0 / 256.0)
                        hi = rows4.tile([1, 512], BF16, tag="hi",
                                        name=f"hi{h2}_{pqb}")
                        nc.vector.tensor_copy(out=hi[:], in_=rcp[:])
                        lo = rows4.tile([1, 512], BF16, tag="lo",
                                        name=f"lo{h2}_{pqb}")
                        nc.vector.tensor_tensor(lo[:], rcp[:], hi[:],
                                                ALU.subtract)
                        pend_sc.append((h2, pqb, hi, lo))
                prev = (cur[0], cur[1], new_ctx) if cur else None

            # --- phase 3: o_proj, obp-outer; drains covered by obp0 -----
            for obp in range(4):
                if pend_sc:
                    emit_sc(pend_sc.pop(0))
                emit_om(obp)
                if pend_fin:
                    emit_fin(pend_fin.pop(0), t1_map)
                for tt in range(16):
                    if obp == 0 and tt in (1, 3, 5) and pend_sc:
                        emit_sc(pend_sc.pop(0))
                    if obp == 0 and tt in (2, 4, 6) and pend_fin:
                        emit_fin(pend_fin.pop(0), t1_map)
                    emit_oproj_obp_tt(obp, tt)

    if split_waits:
        _split_sync_waits(nc)
    return nc


_NC_CACHE = None


def _get_nc():
    global _NC_CACHE
    if _NC_CACHE is None:
        _NC_CACHE = _build()
    return _NC_CACHE


def _binarize(w):
    """Match reference bitnet_linear: s = max(mean|W|_row, 1e-8) (>0), so
    sign(W/s) == sign(W). Returns (sign(W) as fp8, s as f32)."""
    w = np.asarray(w, np.float32)
    s = np.maximum(
        np.abs(w).mean(axis=1, dtype=np.float64).astype(np.float32), 1e-8
    )
    return np.sign(w).astype(NPF8), s


def _pack_w(wt, nf):
    # [4096 in, nf out] -> [128, 2, CC, nf]; in-feature f = cc*256 + i*128 + p
    return np.ascontiguousarray(
        wt.reshape(CC, 2, 128, nf).transpose(2, 1, 0, 3))


def _make_in_maps(hidden_states, q_weight, q_scale, k_weight, k_scale,
                  v_weight, v_scale, o_weight, o_scale):
    hs = np.asarray(hidden_states, np.float32)
    b, t, hid = hs.shape
    assert (b, t, hid) == (1, T, HIDDEN)

    xT = np.ascontiguousarray(hs[0].T)                  # [4096, T] f32
    xq8 = xT.astype(NPF8)
    dx8 = (xT - xq8.astype(np.float32)).astype(NPF8)

    def pack_x(a):
        # [4096, T] -> [TQ, 128, 2, CC, 512]
        return np.ascontiguousarray(
            a.reshape(CC, 2, 128, TQ, 512).transpose(3, 2, 1, 0, 4))

    xq4 = pack_x(xq8)
    dx4 = pack_x(dx8)

    bq, s_q = _binarize(q_weight)
    bk, s_k = _binarize(k_weight)
    bv, s_v = _binarize(v_weight)
    bo, s_o = _binarize(o_weight)

    sq_full = s_q * np.asarray(q_scale, np.float32)                # [4096]
    sk_full = s_k * np.asarray(k_scale, np.float32) / np.float32(
        np.sqrt(DH))                                               # [1024]
    sv_full = s_v * np.asarray(v_scale, np.float32)                # [1024]
    so_full = s_o * np.asarray(o_scale, np.float32)                # [4096]

    ones8 = np.ones((128, 2, DH), NPF8)
    ones128 = np.ones((1, 128), NPBF)
    ident = np.eye(128, dtype=NPBF)

    in_maps = []
    for i in range(N_CORES):
        fq = slice(FQ * i, FQ * (i + 1))
        fk = slice(DH * i, DH * (i + 1))
        # o weights: [4096 o, 512 f] -> [4 obp, 128 p, 2 c, 2 i, 1024 o]
        bo_t = np.ascontiguousarray(bo[:, fq].T)        # [512 f, 4096 o] fp8
        bo_p = np.ascontiguousarray(
            bo_t.reshape(2, 2, 128, 4, 1024).transpose(3, 2, 0, 1, 4))
        sk_c = sk_full[fk].reshape(DH, 1)
        sv_c = sv_full[fk].reshape(DH, 1)
        del sk_c, sv_c
        sk_c = sk_full[fk].reshape(DH, 1)
        sv_c = sv_full[fk].reshape(DH, 1)
        in_maps.append({
            "xq": xq4,
            "dxq": dx4,
            "bqt": _pack_w(np.ascontiguousarray(bq[fq].T), FQ),
            "bkt": _pack_w(np.ascontiguousarray(bk[fk].T), DH),
            "bvt": _pack_w(np.ascontiguousarray(bv[fk].T), DH),
            "bot": bo_p,
            "sq": np.ascontiguousarray(
                sq_full[fq].reshape(H, DH, 1).astype(np.float32)),
            "sk": np.ascontiguousarray(sk_c.astype(np.float32)),
            "sv": np.ascontiguousarray(sv_c.astype(np.float32)),
            "ones8": ones8,
            "ones128": ones128,
            "ident": ident,
        })
    return in_maps, so_full


def kernel(**inputs):
    in_maps, so_full = _make_in_maps(**inputs)
    nc = _get_nc()
    res = run_bass_kernel_spmd(
        nc, in_maps, core_ids=list(range(N_CORES)), trace=TRACE,
        trace_cores=list(range(N_CORES)) if TRACE and TRACE_ALL_CORES else None,
    )
    if TRACE:
        kernel.last_exec_time_ns = res.exec_time_ns
        kernel.last_mean_exec_time_ns = res.mean_exec_time_ns

    y = np.zeros((T, HIDDEN), np.float32)
    om = np.zeros((HIDDEN,), np.float32)
    for i in range(N_CORES):
        y += res.results[i]["y"].astype(np.float32)
        om += np.asarray(res.results[i]["om"], np.float32).reshape(HIDDEN)
    y += om[None, :]
    y *= so_full[None, :] * np.float32(1.0 / 256.0)
    return y.reshape(1, T, HIDDEN)
